# revision 12
# baseline (speedup 1.0000x reference)
"""Bottom-k cross-entropy loss on 8 Trainium2 NeuronCores.

Per-sample CE over [8192, 32000] logits, then mean of the 4096 smallest
losses.  Data-parallel: rows sharded across 8 cores.

Stream: logits quantized host-side to fp8 E3M4 (4 mantissa bits at the
N(0,1) range) -- 32MB/core instead of 131MB -- and the exp+accumulate
runs on TWO engines concurrently over disjoint column ranges:

  - ACT: spline exp with accum_out (~57% of columns).
  - DVE: runtime-registered custom op EXPSQ32_ANT, e^x ~= (c(1+x/32))^32
    as affine + 5 chained squarings + ADD accumulation, 8/8 ALU stages,
    one instruction per element.  The affine constants come in via the
    two scalar operands, so the same op also evaluates the threshold
    exponential e^t for t~11 with re-centered constants
    (e^t = (e^{c0/32} (1 + (t-c0)/32))^32).

Selection exploits  mean_bottom_m = t - (1/m) sum_all relu(t - ce),
exact for t in [ce_(m), ce_(m+1)] and degrading only as ~0.4 delta^2
for |t - ce_(m)| = delta.  t is computed from a 4/8 SAMPLE: blocks 0-3
are all-gathered as bf16 y-values right after block 3, replicated to
all partitions by PE rank-1 matmuls into PSUM banks (ones^T @ row --
no gpsimd ucode reload, ~0.5us/bank), and threshold-counted in two
dyadic rounds on DVE in queue slots that are slack -- all under the
stream.  The final accumulation is PER-CORE LOCAL (ln + relu-accum on
the core's own [128 x 8] y-values + one cross-partition matmul) and a
single-scalar AllReduce, so no post-stream value gather exists and the
collective fabric latency (~10-20us/op here) is paid once on a 4-byte
payload at stream end.
"""

import math
import numpy as np

N_CORES = 8
N_FULL, V_FULL = 8192, 32000
P = 128

# Bracket steps.
S1 = 2.0**-2
S2W = 10.0 * S1 / 128.0  # = 5 * 2^-8, exact dyadic
RB_A = 3  # row blocks in the early (sample) all-gather

# ACT/DVE column split per row block.
A_CHUNK = 9216
D_CHUNK = 6784
A_COLS = 2 * A_CHUNK
D_COLS = 2 * D_CHUNK
assert A_COLS + D_COLS == V_FULL

# EXPSQ32_ANT logit-domain constants: m = s0*x + s1, out = m^32.
EXPSQ_C = 1.00091944
EXPSQ_S0 = EXPSQ_C / 32.0
EXPSQ_S1 = EXPSQ_C
# threshold-domain constants, re-centered at c0 ~ median ce = ln(V e^.5)
_C0T = math.log(V_FULL) + 0.5
_E32 = math.exp(_C0T / 32.0)
EXPT_S0 = _E32 / 32.0
EXPT_S1 = _E32 * (1.0 - _C0T / 32.0)

_EXPSQ_NAME = "EXPSQ32_ANT"


def _register_expsq():
    """Register the custom DVE op in concourse's in-process registry
    (the documented extension point is appending to dve_ops.OPS)."""
    from concourse.dve_ops import (
        OPS,
        CUSTOM_DVE_SPECS,
        DveOp,
        _SUB_OPCODE_FOR_NAME,
        _CUSTOM_DVE_ROW_BASE,
    )
    from concourse.dve_spec import Spec, Src0, C0, C1, lower, AluOp
    from concourse.dve_uop import DveOpSpec

    for op in OPS:
        if op.name == _EXPSQ_NAME:
            return op

    def _ref(in0, in1, s0, s1, imm2):
        m = (in0.astype(np.float32) * np.float32(s0) + np.float32(s1)).astype(
            np.float32
        )
        for _ in range(5):
            m = (m * m).astype(np.float32)
        return m, m.reshape(m.shape[0], -1).sum(axis=-1, keepdims=True).astype(
            np.float32
        )

    m = Src0 * C0 + C1
    for _ in range(5):
        m = m * m
    spec = Spec(body=m, accum=AluOp.ADD, reference=_ref)

    row = _CUSTOM_DVE_ROW_BASE + len(OPS)
    _SUB_OPCODE_FOR_NAME[_EXPSQ_NAME] = row
    shas = {
        ver: DveOpSpec(
            name=_EXPSQ_NAME, opcode=row, uops=lower(spec, ver=ver), rd1_en=False
        ).sha(ver)
        for ver in ("v3", "v4")
    }
    op = DveOp(_EXPSQ_NAME, spec, subdim=False, uops_sha=shas)
    OPS.append(op)
    CUSTOM_DVE_SPECS[_EXPSQ_NAME] = spec
    return op


def build_nc(n_cores, r, v):
    """Build the SPMD Bass program (identical on every core)."""
    from concourse import bass, bacc, mybir, tile

    expsq = _register_expsq()

    assert r % P == 0
    rb_n = r // P
    ng = r * n_cores
    m = ng // 2
    na = RB_A * P * n_cores        # sample size (4096)
    nbank = na // 512              # PSUM banks per counting round (8)
    target = RB_A * m / 8.0        # sample-scaled rank target (2048)
    f32 = mybir.dt.float32
    bf16 = mybir.dt.bfloat16
    f8 = mybir.dt.float8e3
    add_dep = tile.add_dep_helper

    nc = bacc.Bacc()
    x = nc.declare_dram_parameter("x", [r, v], f8, isOutput=False)
    offs = nc.declare_dram_parameter("offs", [P, rb_n], mybir.dt.int32, isOutput=False)
    e1 = nc.declare_dram_parameter("e1", [P, 1], f32, isOutput=False)
    io2 = nc.declare_dram_parameter("io2", [P, 1], f32, isOutput=False)
    out = nc.declare_dram_parameter("out", [1, 1], f32, isOutput=True)

    with tile.TileContext(nc) as tc:
        with (
            tc.tile_pool(name="dram", bufs=1, space="DRAM") as dpool,
            tc.tile_pool(name="consts", bufs=1) as cpool,
            tc.tile_pool(name="xa", bufs=6) as xapool,
            tc.tile_pool(name="xd", bufs=6) as xdpool,
            tc.tile_pool(name="part", bufs=3) as partpool,
            tc.tile_pool(name="sel", bufs=1) as selpool,
            tc.tile_pool(name="psbank", bufs=8, space="PSUM") as bpool,
        ):
            ya_local = dpool.tile([RB_A * P, 1], bf16, name="ya_local")
            ya_all = dpool.tile([na, 1], bf16, addr_space="Shared", name="ya_all")
            db1 = dpool.tile([1, 1], f32, name="db1")
            db2 = dpool.tile([8, 1], f32, addr_space="Shared", name="db2")
            dl_dram = dpool.tile([1, 1], f32, name="dl_dram")
            dfin = dpool.tile([8, 1], f32, addr_space="Shared", name="dfin")

            offs_sb = cpool.tile([P, rb_n], mybir.dt.int32)
            nc.gpsimd.dma_start(offs_sb[:], offs[:])
            e1_sb = cpool.tile([P, 1], f32)
            nc.gpsimd.dma_start(e1_sb[:], e1[:])
            io2_sb = cpool.tile([P, 1], f32)
            nc.gpsimd.dma_start(io2_sb[:], io2[:])

            # dummy all-reduce: syncs the cores right after launch (absorbs
            # launch skew) and warms the collective reduce path used by the
            # final scalar.  Output unread.
            d_sb = cpool.tile([1, 1], f32)
            nc.vector.memset(d_sb[:], 0.0)
            nc.gpsimd.dma_start(db1[:], d_sb[:])
            nc.gpsimd.collective_compute(
                "AllGather",
                mybir.AluOpType.bypass,
                replica_groups=[list(range(n_cores))],
                ins=[db1[:].opt()],
                outs=[db2[:].opt()],
            )

            # gather picked logits: x.flat[row*v + label] for each local row
            picked8 = cpool.tile([P, rb_n], f8)
            x_flat = x[:].rearrange("a b -> (a b) ()")
            for rbi in range(rb_n):
                nc.gpsimd.indirect_dma_start(
                    out=picked8[:, rbi : rbi + 1],
                    out_offset=None,
                    in_=x_flat,
                    in_offset=bass.IndirectOffsetOnAxis(
                        ap=offs_sb[:, rbi : rbi + 1], axis=0
                    ),
                )

            expnp = cpool.tile([P, rb_n], f32)
            ys = cpool.tile([P, rb_n], f32)      # raw per-block sumexp
            ysb = cpool.tile([P, RB_A], bf16)    # sample y in bf16 for gather
            ysy = cpool.tile([P, rb_n], f32)     # y = sumexp*exp(-picked)
            lces = cpool.tile([P, rb_n], f32)    # local ce values
            row_sb = cpool.tile([1, na], bf16)   # gathered sample row
            ones1 = cpool.tile([1, P], bf16)
            nc.vector.memset(ones1[:], 1.0)
            dummy_a = selpool.tile([P, 1], f32)
            dummy_d = selpool.tile([P, 1], f32)
            ones = selpool.tile([P, P], f32)
            nc.vector.memset(ones[:], 1.0)
            cnt1 = selpool.tile([P, nbank], f32)
            cnt2 = selpool.tile([P, nbank], f32)
            ca_a = selpool.tile([P, 1], f32)
            ge1 = selpool.tile([P, 1], f32)
            lo1 = selpool.tile([P, 1], f32)
            arg2 = selpool.tile([P, 1], f32)
            e2 = selpool.tile([P, 1], f32)
            c2a = selpool.tile([P, 1], f32)

            def bcast_banks(tagname):
                """Replicate the gathered sample row into PSUM banks via
                rank-1 PE matmuls (ones^T @ row_chunk)."""
                banks = []
                for i in range(nbank):
                    bk = bpool.tile([P, 512], f32, tag="bank", name=f"{tagname}{i}")
                    nc.tensor.matmul(
                        out=bk[:], lhsT=ones1[:],
                        rhs=row_sb[:, i * 512 : (i + 1) * 512],
                        start=True, stop=True,
                    )
                    banks.append(bk)
                return banks

            def count_banks(banks, dst_cols, thr_ap):
                for i, bk in enumerate(banks):
                    nc.vector.tensor_scalar(
                        out=dummy_d[:].broadcast_to([P, 512]),
                        in0=bk[:],
                        scalar1=thr_ap,
                        scalar2=None,
                        op0=mybir.AluOpType.is_le,
                        op1=mybir.AluOpType.add,
                        accum_out=dst_cols[:, i : i + 1],
                    )

            # streaming pass: all chunk loads on the SP/sync HWDGE ring;
            # ACT chunks (spline exp + accum) and DVE chunks (EXPSQ32
            # custom op + accum) interleave so both engines run
            # concurrently.  Block epilogues are split: the DVE-partial
            # reduce issues immediately; the ACT-dependent reduce issues
            # after the NEXT block's chunks so the DVE queue head never
            # waits on the slightly-slower ACT.
            s_d_t = {}
            parts_a_t = {}
            b4a0 = None

            def emit_epilogue(b):
                """ys[:, b] = (sum parts_a[b]) + s_d[b] (DVE)."""
                s_a = selpool.tile([P, 1], f32, name=f"sa{b}", tag="sblk")
                nc.vector.tensor_reduce(
                    s_a[:], parts_a_t[b][:], axis=mybir.AxisListType.X,
                    op=mybir.AluOpType.add,
                )
                nc.vector.tensor_tensor(
                    out=ys[:, b : b + 1], in0=s_a[:], in1=s_d_t[b][:],
                    op=mybir.AluOpType.add,
                )

            spans = [
                ("a", 0, 0, A_CHUNK),
                ("d", 0, A_COLS, A_COLS + D_CHUNK),
                ("a", 1, A_CHUNK, A_COLS),
                ("d", 1, A_COLS + D_CHUNK, V_FULL),
            ]
            r1_banks = None
            for rbi in range(rb_n):
                parts_a = partpool.tile([P, 2], f32, tag="pa", name=f"pa{rbi}")
                parts_d = partpool.tile([P, 2], f32, tag="pd", name=f"pd{rbi}")
                parts_a_t[rbi] = parts_a

                rows = slice(rbi * P, (rbi + 1) * P)
                for eng, ci, lo, hi in spans:
                    if eng == "a":
                        xt = xapool.tile([P, hi - lo], f8, tag="xa")
                    else:
                        xt = xdpool.tile([P, hi - lo], f8, tag="xd")
                    nc.sync.dma_start(xt[:], x[rows, lo:hi])
                    if eng == "a":
                        ai = nc.scalar.activation(
                            out=dummy_a[:].broadcast_to([P, hi - lo]),
                            in_=xt[:],
                            func=mybir.ActivationFunctionType.Exp,
                            accum_out=parts_a[:, ci : ci + 1],
                        )
                        if rbi == 4 and ci == 0:
                            b4a0 = ai.ins
                    else:
                        nc.vector._custom_dve(
                            expsq,
                            out=dummy_d[:].broadcast_to([P, hi - lo]),
                            in0=xt[:],
                            s0=EXPSQ_S0,
                            s1=EXPSQ_S1,
                            accum_out=parts_d[:, ci : ci + 1],
                        )

                # DVE-partial reduce for this block
                s_d = selpool.tile([P, 1], f32, name=f"sd{rbi}", tag="sblk2")
                nc.vector.tensor_reduce(
                    s_d[:], parts_d[:], axis=mybir.AxisListType.X,
                    op=mybir.AluOpType.add,
                )
                s_d_t[rbi] = s_d

                # previous block's ACT-dependent epilogue
                if rbi > 0:
                    emit_epilogue(rbi - 1)

                if rbi == RB_A:
                    # sample stage: y = sumexp*exp(-picked) for blocks
                    # 0..RB_A-1 in bf16, all-gather, row load (sync ring).
                    # exp(-picked) on ACT first (exact spline; pinned after
                    # block 4's first chunk so it never stalls the stream).
                    ei = nc.scalar.activation(
                        out=expnp[:], in_=picked8[:],
                        func=mybir.ActivationFunctionType.Exp, scale=-1.0,
                    )
                    add_dep(ei.ins, b4a0, sync=False, reason="expnp after b4a0")
                    nc.vector.tensor_tensor(
                        out=ysb[:], in0=ys[:, :RB_A], in1=expnp[:, :RB_A],
                        op=mybir.AluOpType.mult,
                    )
                    nc.gpsimd.dma_start(
                        ya_local[:].rearrange("(p b) 1 -> p b", b=RB_A), ysb[:]
                    )
                    nc.gpsimd.collective_compute(
                        "AllGather",
                        mybir.AluOpType.bypass,
                        replica_groups=[list(range(n_cores))],
                        ins=[ya_local[:].opt()],
                        outs=[ya_all[:].opt()],
                    )
                    # row load on gpsimd/SWDGE: it waits on the collective,
                    # and on the stream (sync) ring that wait head-blocks
                    # the ring and starves the last blocks' chunk loads
                    nc.gpsimd.dma_start(
                        row_sb[:], ya_all[:].rearrange("a 1 -> 1 a")
                    )

                if rbi == rb_n - 2:
                    # PE-broadcast the sample into PSUM banks for round 1
                    # (PE idle; row lands ~40us before this is consumed)
                    r1_banks = bcast_banks("r1b")

            # ---- all stream chunks issued: threshold rounds 1+2 ----
            count_banks(r1_banks, cnt1, e1_sb[:])
            nc.vector.tensor_reduce(
                ca_a[:], cnt1[:], axis=mybir.AxisListType.X,
                op=mybir.AluOpType.add,
            )
            nc.vector.tensor_scalar(
                out=ge1[:], in0=ca_a[:], scalar1=target,
                scalar2=None, op0=mybir.AluOpType.is_ge,
            )
            g1 = r1_banks[0][:, 0:1]
            nc.tensor.matmul(out=g1, lhsT=ones[:], rhs=ge1[:], start=True, stop=True)
            nc.vector.tensor_scalar(
                out=lo1[:], in0=g1, scalar1=-S1, scalar2=None,
                op0=mybir.AluOpType.mult,
            )
            nc.vector.tensor_tensor(
                out=arg2[:], in0=lo1[:], in1=io2_sb[:], op=mybir.AluOpType.add
            )
            nc.vector._custom_dve(
                expsq, out=e2[:], in0=arg2[:],
                s0=EXPT_S0, s1=EXPT_S1, accum_out=None,
            )
            r2_banks = bcast_banks("r2b")
            count_banks(r2_banks, cnt2, e2[:])
            nc.vector.tensor_reduce(
                c2a[:], cnt2[:], axis=mybir.AxisListType.X, op=mybir.AluOpType.add
            )
            ge2 = selpool.tile([P, 1], f32)
            nc.vector.tensor_scalar(
                out=ge2[:], in0=c2a[:], scalar1=target, scalar2=None,
                op0=mybir.AluOpType.is_ge,
            )
            g2 = r2_banks[0][:, 0:1]
            nc.tensor.matmul(out=g2, lhsT=ones[:], rhs=ge2[:], start=True, stop=True)
            lo2 = selpool.tile([P, 1], f32)
            nc.vector.tensor_scalar(
                out=lo2[:], in0=g2, scalar1=-S2W, scalar2=lo1[:],
                op0=mybir.AluOpType.mult, op1=mybir.AluOpType.add,
            )
            # final threshold t = first round-2 grid point with count>=target
            c_t = 124.0 * S1 + 129.0 * S2W
            tf = selpool.tile([P, 1], f32)
            nc.vector.tensor_scalar(
                out=tf[:], in0=lo2[:], scalar1=c_t, scalar2=None,
                op0=mybir.AluOpType.add,
            )

            # local accumulation: ce = ln(sumexp*exp(-picked)) for this
            # core's 1024 rows, then sum relu(t - ce)
            emit_epilogue(rb_n - 1)
            nc.vector.tensor_tensor(
                out=ysy[:], in0=ys[:], in1=expnp[:], op=mybir.AluOpType.mult
            )
            ln_i = nc.scalar.activation(
                out=lces[:], in_=ysy[:], func=mybir.ActivationFunctionType.Ln,
            )
            srl = selpool.tile([P, 1], f32)
            relu_i = nc.scalar.activation(
                out=dummy_a[:].broadcast_to([P, rb_n]),
                in_=lces[:],
                func=mybir.ActivationFunctionType.Relu,
                bias=tf[:],
                scale=-1.0,
                accum_out=srl[:],
            )
            add_dep(relu_i.ins, ln_i.ins, sync=False, reason="relu after ln")
            g3 = r2_banks[1][:, 0:1]
            nc.tensor.matmul(out=g3, lhsT=ones[:], rhs=srl[:], start=True, stop=True)
            dl_sb = selpool.tile([P, 1], f32)
            nc.vector.tensor_scalar(
                out=dl_sb[:], in0=g3, scalar1=1.0, scalar2=None,
                op0=mybir.AluOpType.mult,
            )
            nc.gpsimd.dma_start(dl_dram[:], dl_sb[0:1, :])
            nc.gpsimd.collective_compute(
                "AllGather",
                mybir.AluOpType.bypass,
                replica_groups=[list(range(n_cores))],
                ins=[dl_dram[:].opt()],
                outs=[dfin[:].opt()],
            )
            dfin_sb = selpool.tile([1, 8], f32)
            nc.gpsimd.dma_start(dfin_sb[:], dfin[:].rearrange("a 1 -> 1 a"))
            dsum = selpool.tile([1, 1], f32)
            nc.vector.tensor_reduce(
                dsum[:], dfin_sb[:], axis=mybir.AxisListType.X,
                op=mybir.AluOpType.add,
            )
            # res = t - (sum_global relu)/m
            res = selpool.tile([1, 1], f32)
            nc.vector.tensor_scalar(
                out=res[:], in0=dsum[:], scalar1=-1.0 / m,
                scalar2=tf[0:1, :], op0=mybir.AluOpType.mult,
                op1=mybir.AluOpType.add,
            )
            nc.sync.dma_start(out[:], res[:])

    if not nc.is_finalized():
        nc.finalize()
    return nc


def make_host_inputs(x_full, labels_full, n_cores, r, v):
    """Shard rows across cores, quantize to fp8 E3M4, build input maps."""
    import ml_dtypes

    rb_n = r // P
    e1 = np.exp((np.arange(P, dtype=np.float64) + 1) * S1).astype(np.float32)
    io2 = (124 * S1 + (np.arange(P, dtype=np.float64) + 1) * S2W).astype(np.float32)
    in_maps = []
    for c in range(n_cores):
        rows = slice(c * r, (c + 1) * r)
        xs = np.ascontiguousarray(x_full[rows], dtype=np.float32).astype(
            ml_dtypes.float8_e3m4
        )
        lb = np.asarray(labels_full[rows], dtype=np.int64)
        offs_flat = (np.arange(r, dtype=np.int64) * v + lb).astype(np.int32)
        offs = np.ascontiguousarray(offs_flat.reshape(rb_n, P).T)
        in_maps.append(
            {
                "x": xs,
                "offs": offs,
                "e1": e1.reshape(P, 1),
                "io2": io2.reshape(P, 1),
            }
        )
    return in_maps


def run(inputs, trace=False):
    from concourse.bass_utils import run_bass_kernel_spmd

    x_full = np.asarray(inputs["outputs"], dtype=np.float32)
    labels_full = np.asarray(inputs["labels"])
    n, v = x_full.shape
    r = n // N_CORES
    nc = build_nc(N_CORES, r, v)
    in_maps = make_host_inputs(x_full, labels_full, N_CORES, r, v)
    try:
        res = run_bass_kernel_spmd(
            nc, in_maps, list(range(N_CORES)), trace=trace
        )
    except Exception:
        # transient device errors (e.g. a wedged core from a prior run)
        # usually clear on retry
        res = run_bass_kernel_spmd(
            nc, in_maps, list(range(N_CORES)), trace=trace
        )
    val = np.asarray(res.results[0]["out"], dtype=np.float32).reshape(-1)[0]
    return np.asarray(val, dtype=np.float32), res


def kernel(outputs=None, labels=None, **_ignored):
    out, _ = run({"outputs": outputs, "labels": labels})
    return out
